# revision 12
# baseline (speedup 1.0000x reference)
"""Trainium2 Bass kernel for masked multi-head attention (nn_Attention_25271587569919).

Problem: B=4, S=2048, D=1024, 16 heads x 64. out = (softmax(QK^T/8 + pad/causal mask) V) WO.

Sharding: 8 cores = 4 batches x 2 head-groups (Megatron-style). Each core computes,
for its batch b and its 8 heads:
    QT/KT = (X Wq_g)^T in [dh, seq] layout,  V = X Wv_g in [seq, dh] layout,
    S^T tiles [k, q] (so pad mask = per-partition ACT bias, no transposes anywhere),
    P = exp(S^T/8 + pad) (no max subtraction; scores are O(1)),
    causal handled by narrowing the q-range of diagonal-straddling tiles plus one
    128x128 triangular 0/1 multiply on the diagonal block,
    ctx^T (+rowsum via an all-ones 65th column of V) = Vaug^T @ P accumulated in PSUM,
    ctx^T normalized by 1/rowsum (reciprocal of the rowsum row, broadcast across
    partitions with a K=2 ones-matmul), then out_partial = ctx @ Wo_g.
Host sums the two head-group partials per batch. No collectives needed.

The emission is software-pipelined over q-stripes j: projection work for later
stripes and deferred output projections are interleaved between the attention
i-iterations, and scores for i+1 are emitted before the PV matmuls of i so the
TensorEngine never sits behind the exp latency.

Scheduling notes (from perfetto analysis):
  - the two per-head-pair scores matmuls (K=64) auto-row-tile via base
    partitions 0/64 and run CONCURRENTLY on the PE (pair ~= one N-cycle pass);
  - a short chain of throwaway warm-up matmuls runs while the first DMA
    chunks land, so the PE's DVFS ramp is already hot when real work arrives;
  - stripe 0 computes only the minimal prefix (Q slab 0, K slab 0, V) before
    attention starts; Q/K slabs 1-3 are force-popped at the hp block starts,
    so the first exp fires ~20us earlier than a full-projection prologue;
  - startup loads are chunked per-kt across THREE DMA trigger queues in strict
    consumption order (thin 128-col W slabs first); every steady-state load is
    also chunked so no 6us monolithic descriptor-gen ever blocks a queue;
  - interleave units are 2-matmul quarters paced by a fractional credit;
  - the norm's rowsum-gather/shift DMAs issue right at PSUM evacuation, the
    compute half (broadcast matmul, reciprocal, scale) is deferred, so the
    gathers' ~2us DMA latency is always covered by attention iterations;
  - the very last normalization replaces its DMAs with selector matmuls
    (rowsum broadcast + head-B shift on the PE) so the final WO block is not
    gated on SBUF-DMA latency at the tail;
  - WO fillers are deferred to the late, exp-throughput-bound stripes;
  - output staged bf16 (summed f32 host-side) to halve the final DMA drain.
"""

import numpy as np
import ml_dtypes

BF = ml_dtypes.bfloat16
S = 2048
D = 1024
HG = 512          # head-group width (8 heads x 64)
DH = 64
NKT = 16          # seq tiles of 128 (k side)
NQT = 4           # seq tiles of 512 (q side)
NEG = -30000.0

_CACHE = {}


def _build():
    import concourse.bass as bass  # noqa: F401
    import concourse.tile as tile
    from concourse import bacc, mybir

    f32 = mybir.dt.float32
    bf16 = mybir.dt.bfloat16
    Exp = mybir.ActivationFunctionType.Exp

    nc = bacc.Bacc("TRN2", target_bir_lowering=False, debug=False, num_devices=8)

    xq_d = nc.dram_tensor("xq", [D, S], bf16, kind="ExternalInput")
    xk_d = nc.dram_tensor("xk", [D, S], bf16, kind="ExternalInput")
    xv_d = nc.dram_tensor("xv", [D, S], bf16, kind="ExternalInput")
    wq_d = nc.dram_tensor("wq", [D, HG], bf16, kind="ExternalInput")
    wk_d = nc.dram_tensor("wk", [D, HG], bf16, kind="ExternalInput")
    wv_d = nc.dram_tensor("wv", [D, HG], bf16, kind="ExternalInput")
    wo_d = nc.dram_tensor("wo", [HG, D], bf16, kind="ExternalInput")
    padb_d = nc.dram_tensor("padb", [128, NKT], f32, kind="ExternalInput")
    trim_d = nc.dram_tensor("trim", [128, 128], bf16, kind="ExternalInput")
    ones2_d = nc.dram_tensor("ones2", [2, 128], bf16, kind="ExternalInput")
    sel_d = nc.dram_tensor("sel", [65, 320], bf16, kind="ExternalInput")
    out_d = nc.dram_tensor("out", [S, D], bf16, kind="ExternalOutput")

    with tile.TileContext(nc) as tc:
        with (
            tc.tile_pool(name="consts", bufs=1) as consts,
            tc.tile_pool(name="big", bufs=1) as big,
            tc.tile_pool(name="xpool", bufs=6) as xpool,
            tc.tile_pool(name="ppool", bufs=4) as ppool,
            tc.tile_pool(name="cspool", bufs=6) as cspool,
            tc.tile_pool(name="rspool", bufs=4) as rspool,
            tc.tile_pool(name="tmppool", bufs=3) as tmppool,
            tc.tile_pool(name="outpool", bufs=3) as outpool,
            tc.tile_pool(name="pspool", bufs=2, space="PSUM") as pspool,
            tc.tile_pool(name="psupool", bufs=2, space="PSUM") as psupool,
            tc.tile_pool(name="ctxpool", bufs=2, space="PSUM") as ctxpool,
        ):
            wq_sb = consts.tile([128, 8, HG], bf16, tag="wq")
            wk_sb = consts.tile([128, 8, HG], bf16, tag="wk")
            wv_sb = consts.tile([128, 8, HG], bf16, tag="wv")
            wo_sb = consts.tile([128, 4, D], bf16, tag="wo")
            padb_sb = consts.tile([128, NKT], f32, tag="padb")
            trim_sb = consts.tile([128, 2, 128], bf16, tag="trim")
            ones2_sb = consts.tile([2, 128], bf16, tag="ones2")
            sel_sb = consts.tile([65, 320], bf16, tag="sel")
            scr_sb = consts.tile([128, 512], bf16, tag="scr")

            qt_sb = big.tile([128, 4, S], bf16, tag="qt")    # (X Wq)^T : rows = dh
            kt_sb = big.tile([128, 4, S], bf16, tag="kt")
            vaug_sb = big.tile([128, NKT, 8 * 65], bf16, tag="vaug")  # V + ones col
            ctxt_sb = big.tile([128, 4, S], bf16, tag="ctxt")
            vaug_h = vaug_sb.rearrange("p m (h e) -> p m h e", e=65)

            # ---- warm-up: ~10 throwaway matmuls keep the PE busy while the
            # first DMA chunks land, so the DVFS ramp is hot for real work
            nc.vector.memset(scr_sb, 0.25)
            nc.vector.memset(vaug_h[:, :, :, 64:65], 1.0)
            wup = psupool.tile([128, 512], f32, tag="psu")
            for _ in range(10):
                nc.tensor.matmul(wup, lhsT=scr_sb[:, 0:128], rhs=scr_sb,
                                 start=True, stop=True)

            def load_consts():
                nc.gpsimd.dma_start(out=padb_sb, in_=padb_d.ap())
                nc.gpsimd.dma_start(out=trim_sb[:, 0, :], in_=trim_d.ap())
                nc.gpsimd.dma_start(out=trim_sb[:, 1, :], in_=trim_d.ap())
                nc.gpsimd.dma_start(out=ones2_sb, in_=ones2_d.ap())
                nc.gpsimd.dma_start(out=sel_sb, in_=sel_d.ap())

            def load_w_cols(dst, dram, kt, c0, c1, q):
                def emit():
                    q.dma_start(
                        out=dst[:, kt, c0:c1],
                        in_=dram.ap().rearrange(
                            "(kt p) n -> p kt n", p=128)[:, kt, c0:c1])
                return emit

            def load_wo_chunk(kt, q):
                def emit():
                    ktt, h = kt // 2, kt % 2
                    cols = slice(512 * h, 512 * (h + 1))
                    q.dma_start(
                        out=wo_sb[:, ktt, cols],
                        in_=wo_d.ap().rearrange(
                            "(kt p) n -> p kt n", p=128)[:, ktt, cols])
                return emit

            # ---------- x loads (chunked) + projections per q/seq stripe ----------
            x_tiles = {}

            def alloc_x(j, name):
                t = xpool.tile([128, 8, 512], bf16, tag="x")
                x_tiles[(j, name)] = t

            def load_x_2kt(j, name, dram, kt2, q):
                qs = slice(512 * j, 512 * (j + 1))

                def emit():
                    q.dma_start(
                        out=x_tiles[(j, name)][:, 2 * kt2:2 * kt2 + 2, :],
                        in_=dram.ap().rearrange(
                            "(kt p) s -> p kt s", p=128)[:, 2 * kt2:2 * kt2 + 2, qs])
                return emit

            def load_x_1kt(j, name, dram, kt, q):
                qs = slice(512 * j, 512 * (j + 1))

                def emit():
                    q.dma_start(
                        out=x_tiles[(j, name)][:, kt:kt + 1, :],
                        in_=dram.ap().rearrange(
                            "(kt p) s -> p kt s", p=128)[:, kt:kt + 1, qs])
                return emit

            def a_loads(j):
                # DMA issues only (no PE cost): emitted inline at the start of
                # the PREVIOUS stripe's attention so transfers overlap it
                alloc_x(j, "xq"); alloc_x(j, "xk"); alloc_x(j, "xv")
                units = []
                for kt2 in range(4):
                    units.append(load_x_2kt(j, "xq", xq_d, kt2,
                                            nc.sync if kt2 % 2 == 0 else nc.gpsimd))
                for kt2 in range(4):
                    units.append(load_x_2kt(j, "xk", xk_d, kt2,
                                            nc.gpsimd if kt2 % 2 == 0 else nc.sync))
                for kt2 in range(4):
                    units.append(load_x_2kt(j, "xv", xv_d, kt2,
                                            nc.sync if kt2 % 2 == 0 else nc.gpsimd))
                return units

            hold = {}

            def proj_t(w_sb, dst_sb, j, t, x_name, part):
                # 2-matmul quarters so the interleaver can fill the ~400ns
                # exp-latency bubbles; the PSUM tile carries across parts
                qs = slice(512 * j, 512 * (j + 1))

                def emit():
                    key = (j, x_name, t)
                    if part == 0:
                        ps_new = psupool.tile([128, 512], f32, tag="psu")
                        hold[key] = ps_new
                    ps = hold[key]
                    for kt in range(2 * part, 2 * part + 2):
                        nc.tensor.matmul(
                            ps,
                            lhsT=w_sb[:, kt, 128 * t:128 * (t + 1)],
                            rhs=x_tiles[(j, x_name)][:, kt, :],
                            start=(kt == 0), stop=(kt == 7),
                        )
                    if part == 3:
                        nc.vector.tensor_copy(out=dst_sb[:, t, qs], in_=ps)
                return emit

            def proj_v(j, m, part):
                def emit():
                    key = (j, "v", m)
                    if part == 0:
                        ps_new = psupool.tile([128, 512], f32, tag="psu")
                        hold[key] = ps_new
                    ps = hold[key]
                    for kt in range(2 * part, 2 * part + 2):
                        nc.tensor.matmul(
                            ps,
                            lhsT=x_tiles[(j, "xv")][:, kt, 128 * (m - 4 * j):128 * (m - 4 * j + 1)],
                            rhs=wv_sb[:, kt, :],
                            start=(kt == 0), stop=(kt == 7),
                        )
                    if part == 3:
                        nc.vector.tensor_copy(
                            out=vaug_h[:, m, :, 0:64],
                            in_=ps.rearrange("p (h e) -> p h e", e=64),
                        )
                return emit

            def a_compute(j):
                units = []
                for t in range(4):
                    for part in range(4):
                        units.append(proj_t(wq_sb, qt_sb, j, t, "xq", part))
                for t in range(4):
                    for part in range(4):
                        units.append(proj_t(wk_sb, kt_sb, j, t, "xk", part))
                for m in range(4 * j, 4 * j + 4):
                    for part in range(4):
                        units.append(proj_v(j, m, part))
                return units

            # ---------- stripe-C: output projection for q stripe j ----------
            def c_units(j):
                holder = {}

                def wo_mn(m, n, half):
                    def emit():
                        if half == 0:
                            ps_new = psupool.tile([128, 512], f32, tag="psu")
                            holder[(m, n)] = ps_new
                        ps = holder[(m, n)]
                        for kt in (0, 1) if half == 0 else (2, 3):
                            nc.tensor.matmul(
                                ps,
                                lhsT=ctxt_sb[:, kt, 128 * m:128 * (m + 1)],
                                rhs=wo_sb[:, kt, 512 * n:512 * (n + 1)],
                                start=(kt == 0), stop=(kt == 3),
                            )
                        if half == 1:
                            # bf16 staging halves the output bytes: the two
                            # per-core partials are summed in f32 on the host
                            o = outpool.tile([128, 512], bf16, tag="o")
                            nc.vector.tensor_copy(out=o, in_=ps)
                            nc.sync.dma_start(
                                out=out_d.ap()[128 * m:128 * (m + 1),
                                               512 * n:512 * (n + 1)], in_=o)
                    return emit
                return [wo_mn(m, n, half)
                        for m in range(4 * j, 4 * j + 4)
                        for n in range(2) for half in range(2)]

            # ---------- normalization ----------
            # phase 1 (at evacuation): rowsum-gather + head-B shift DMAs issue
            # immediately so their ~2us latency hides under later iterations.
            # phase 2 (deferred): broadcast matmul + reciprocal + scales.
            pending_norm = []

            def norm_phase1(cs_a, cs_b):
                rs = rspool.tile([2, 512], bf16, tag="rs")
                csb2 = tmppool.tile([128, 512], bf16, tag="tmp")
                nc.gpsimd.dma_start(out=rs[0:1, :], in_=cs_a[64:65, :])
                nc.gpsimd.dma_start(out=rs[1:2, :], in_=cs_b[64:65, :])
                nc.gpsimd.dma_start(out=csb2[64:128, :], in_=cs_b[0:64, :])
                return rs, csb2

            def make_norm2(cs_a, cs_b, rs, csb2, hp, j):
                qs = slice(512 * j, 512 * (j + 1))

                def emit():
                    bc = psupool.tile([128, 512], f32, tag="psu")
                    nc.tensor.matmul(bc, lhsT=ones2_sb, rhs=rs,
                                     start=True, stop=True)
                    rb = rspool.tile([128, 512], f32, tag="rb")
                    nc.vector.reciprocal_approx_fast(rb, bc)
                    nc.vector.tensor_mul(
                        ctxt_sb[0:64, hp, qs], cs_a[0:64, :], rb[0:64, :])
                    nc.vector.tensor_mul(
                        ctxt_sb[64:128, hp, qs], csb2[64:128, :], rb[64:128, :])
                return emit

            def norm2_tail(cs_a, cs_b, hp, j):
                # DMA-free variant for the very last normalization: rowsum
                # broadcast and head-B shift run as selector matmuls on the
                # PE so the final WO block is not gated on DMA latency
                qs = slice(512 * j, 512 * (j + 1))
                rbps = psupool.tile([128, 512], f32, tag="psu")
                nc.tensor.matmul(rbps, lhsT=sel_sb[:, 0:128], rhs=cs_a,
                                 start=True, stop=False)
                nc.tensor.matmul(rbps, lhsT=sel_sb[:, 128:256], rhs=cs_b,
                                 start=False, stop=True)
                csps = psupool.tile([128, 512], f32, tag="psu")
                nc.tensor.matmul(csps[64:128, :], lhsT=sel_sb[:, 256:320],
                                 rhs=cs_b, start=True, stop=True)
                rb = rspool.tile([128, 512], f32, tag="rb")
                nc.vector.reciprocal_approx_fast(rb, rbps)
                nc.vector.tensor_mul(
                    ctxt_sb[0:64, hp, qs], cs_a[0:64, :], rb[0:64, :])
                nc.vector.tensor_mul(
                    ctxt_sb[64:128, hp, qs], csps[64:128, :], rb[64:128, :])

            # ---------- stripe-B: attention for q stripe j, with interleave ----------
            def b_stripe(j, queue, hp_force=0, i_force=0):
                ni = 4 * j + 4
                tot_slots = 2 * 4 * ni
                pace = len(queue) / tot_slots if queue else 0.0
                credit = 0.0

                def lo(i):
                    return max(0, (i - 4 * j) * 128)

                def pop_filler(n):
                    for _ in range(n):
                        if pending_norm and queue and queue[0][0] == "c":
                            pending_norm.pop(0)()
                        elif queue:
                            queue.pop(0)[1]()
                        else:
                            break

                for hp in range(4):
                    if hp > 0 and hp_force:
                        # stripe 0: Q/K slab hp must be projected before this
                        # hp block's scores read it (in-order PE queue)
                        pop_filler(hp_force)
                    h0, h1 = 2 * hp, 2 * hp + 1
                    ctx_a = ctxpool.tile([65, 512], f32, tag="ctx")
                    ctx_b = ctxpool.tile([65, 512], f32, tag="ctx")
                    sps = {}

                    def scores(i):
                        c = lo(i)
                        ks = slice(128 * i, 128 * (i + 1))
                        sp = pspool.tile([128, 2, 512], f32, tag="ps")
                        nc.tensor.matmul(
                            sp[:, 0, c:], lhsT=kt_sb[0:64, hp, ks],
                            rhs=qt_sb[0:64, hp, 512 * j + c:512 * (j + 1)],
                            start=True, stop=True)
                        nc.tensor.matmul(
                            sp[:, 1, c:], lhsT=kt_sb[64:128, hp, ks],
                            rhs=qt_sb[64:128, hp, 512 * j + c:512 * (j + 1)],
                            start=True, stop=True)
                        sps[i] = sp

                    scores(0)
                    for i in range(ni):
                        if i + 1 < ni:
                            scores(i + 1)
                        c = lo(i)
                        sp = sps.pop(i)
                        p = ppool.tile([128, 2, 512], bf16, tag="p")
                        nc.scalar.activation(
                            out=p[:, :, c:], in_=sp[:, :, c:], func=Exp,
                            bias=padb_sb[:, i:i + 1], scale=0.125)
                        if i >= 4 * j:
                            nc.vector.tensor_mul(
                                p[:, :, c:c + 128], p[:, :, c:c + 128], trim_sb)

                        def insert_filler(force=False):
                            nonlocal credit
                            credit += pace
                            if force and credit < 1.0 and queue:
                                credit = 1.0
                            while credit >= 1.0 and queue:
                                credit -= 1.0
                                pop_filler(1)

                        if i == 3 and pending_norm:
                            pending_norm.pop(0)()
                        if i_force and hp == 0:
                            # stripe 0: V slab i must be projected before this
                            # iteration's PV matmuls read it
                            pop_filler(i_force)
                        insert_filler(force=(i == 0))
                        nc.tensor.matmul(
                            ctx_a[:, c:], lhsT=vaug_sb[:, i, 65 * h0:65 * h0 + 65],
                            rhs=p[:, 0, c:],
                            start=(i == 0), stop=(i == ni - 1))
                        nc.tensor.matmul(
                            ctx_b[:, c:], lhsT=vaug_sb[:, i, 65 * h1:65 * h1 + 65],
                            rhs=p[:, 1, c:],
                            start=(i == 0), stop=(i == ni - 1))
                        insert_filler()
                    # evacuate PSUM immediately (frees the bank for hp+1)
                    cs_a = cspool.tile([65, 512], bf16, tag="cs")
                    nc.vector.tensor_copy(out=cs_a, in_=ctx_a)
                    cs_b = cspool.tile([65, 512], bf16, tag="cs")
                    nc.vector.tensor_copy(out=cs_b, in_=ctx_b)
                    if j == 3 and hp == 3:
                        norm2_tail(cs_a, cs_b, hp, j)
                    else:
                        rs, csb2 = norm_phase1(cs_a, cs_b)
                        pending_norm.append(make_norm2(cs_a, cs_b, rs, csb2, hp, j))

            # ---------- emit the pipeline ----------
            qs3 = [nc.sync, nc.gpsimd, nc.scalar]
            qi = 0

            def q3():
                nonlocal qi
                q = qs3[qi % 3]
                qi += 1
                return q

            # stripe-0 loads, strict consumption order: thin W col-slabs for
            # the t0 projections first, then the full V/W tails
            alloc_x(0, "xq"); alloc_x(0, "xk"); alloc_x(0, "xv")
            for kt in range(8):
                load_w_cols(wq_sb, wq_d, kt, 0, 128, q3())()
                load_x_1kt(0, "xq", xq_d, kt, q3())()
            load_consts()
            for kt in range(8):
                load_w_cols(wk_sb, wk_d, kt, 0, 128, q3())()
                load_x_1kt(0, "xk", xk_d, kt, q3())()
            for kt2 in range(4):
                load_x_2kt(0, "xv", xv_d, kt2, q3())()
                load_w_cols(wv_sb, wv_d, 2 * kt2, 0, HG, q3())()
                load_w_cols(wv_sb, wv_d, 2 * kt2 + 1, 0, HG, q3())()
            for kt in range(8):
                load_w_cols(wq_sb, wq_d, kt, 128, HG, q3())()
                load_w_cols(wk_sb, wk_d, kt, 128, HG, q3())()
            for kt in range(8):
                load_wo_chunk(kt, q3())()

            # stripe-0 minimal compute prefix: Q slab 0 and K slab 0 only --
            # everything else (V for PV, Q/K slabs 1-3 for hp1-3) is deferred
            # into the stripe-0 queue and force-popped just in time, so hp0's
            # scores/exp run while the V data is still in flight
            for part in range(4):
                proj_t(wq_sb, qt_sb, 0, 0, "xq", part)()
            for part in range(4):
                proj_t(wk_sb, kt_sb, 0, 0, "xk", part)()

            s0_rest = []
            for m in range(4):
                for part in range(4):
                    s0_rest.append(("a", proj_v(0, m, part)))
            for t in range(1, 4):
                for part in range(4):
                    s0_rest.append(("a", proj_t(wq_sb, qt_sb, 0, t, "xq", part)))
                for part in range(4):
                    s0_rest.append(("a", proj_t(wk_sb, kt_sb, 0, t, "xk", part)))

            filler = {}
            for jn in (1, 2, 3):
                filler[jn - 1] = [("a", u) for u in a_compute(jn)]
            filler[0] = s0_rest + filler[0]
            filler[3] = [("c", u) for u in c_units(0)] + \
                        [("c", u) for u in c_units(1)] + \
                        [("c", u) for u in c_units(2)]

            for j in range(NQT):
                if j < NQT - 1:
                    for u in a_loads(j + 1):
                        u()
                queue = filler[j]
                b_stripe(j, queue, hp_force=(8 if j == 0 else 0),
                         i_force=(4 if j == 0 else 0))
                if j < NQT - 1:
                    while pending_norm and len(pending_norm) > 1:
                        pending_norm.pop(0)()
                    for _, u in queue:
                        u()
                else:
                    # tail: a few leftover fillers keep the PE hot while the
                    # inline hp3 norm completes, then the last WO drains
                    leftover = list(queue)
                    while pending_norm:
                        pending_norm.pop(0)()
                    for _ in range(4):
                        if leftover:
                            leftover.pop(0)[1]()
                    c3 = c_units(3)
                    for idx, u in enumerate(c3):
                        u()
                        if idx % 2 == 1 and leftover:
                            leftover.pop(0)[1]()
                    for _, u in leftover:
                        u()

    nc.compile()
    return nc


def _make_trim():
    p = np.arange(128)[:, None]
    f = np.arange(128)[None, :]
    return (f >= p).astype(np.float32).astype(BF)


def kernel(Q_emb, K_emb, V_emb, Q_ini, K_ini, WQ, WK, WV, WO):
    from concourse.bass_utils import run_bass_kernel_spmd

    if "nc" not in _CACHE:
        _CACHE["nc"] = _build()
    nc = _CACHE["nc"]

    Q_emb = np.asarray(Q_emb, np.float32)
    K_emb = np.asarray(K_emb, np.float32)
    V_emb = np.asarray(V_emb, np.float32)
    K_ini = np.asarray(K_ini)
    WQ = np.asarray(WQ, np.float32)
    WK = np.asarray(WK, np.float32)
    WV = np.asarray(WV, np.float32)
    WO = np.asarray(WO, np.float32)

    trim = _make_trim()
    ones2 = np.zeros((2, 128), np.float32)
    ones2[0, 0:64] = 1.0
    ones2[1, 64:128] = 1.0
    ones2 = ones2.astype(BF)
    # sel = [eA | eB | shift] for the DMA-free tail normalization:
    # eA broadcasts cs_a row 64 to out rows 0:64, eB broadcasts cs_b row 64
    # to out rows 64:128, shift moves cs_b rows 0:64 to out rows 64:128.
    sel = np.zeros((65, 320), np.float32)
    sel[64, 0:64] = 1.0
    sel[64, 192:256] = 1.0
    sel[np.arange(64), 256 + np.arange(64)] = 1.0
    sel = sel.astype(BF)
    in_maps = []
    for c in range(8):
        b, g = c // 2, c % 2
        gs = slice(HG * g, HG * (g + 1))
        padb = np.where(K_ini[b] != 0, 0.0, NEG).astype(np.float32)
        if padb[0] != 0.0:
            # key 0 masked would make causal row 0 fully masked -> rowsum 0 ->
            # NaN. The reference emits an (arbitrary) softmax over masked
            # scores there; keep key 0 live so output stays finite.
            padb[0] = 0.0
        in_maps.append({
            "xq": Q_emb[b].T.astype(BF),
            "xk": K_emb[b].T.astype(BF),
            "xv": V_emb[b].T.astype(BF),
            "wq": WQ[:, gs].astype(BF),
            "wk": WK[:, gs].astype(BF),
            "wv": WV[:, gs].astype(BF),
            "wo": WO[gs, :].astype(BF),
            "padb": padb.reshape(NKT, 128).T.copy(),
            "trim": trim,
            "ones2": ones2,
            "sel": sel,
        })

    _CACHE["in_maps"] = in_maps
    res = run_bass_kernel_spmd(nc, in_maps, list(range(8)))
    parts = [res.results[c]["out"].astype(np.float32) for c in range(8)]
    out = np.stack([parts[2 * b] + parts[2 * b + 1] for b in range(4)])
    return out.astype(np.float32)


# revision 14
# speedup vs baseline: 1.0488x; 1.0488x over previous
"""Trainium2 Bass kernel for masked multi-head attention (nn_Attention_25271587569919).

Problem: B=4, S=2048, D=1024, 16 heads x 64. out = (softmax(QK^T/8 + pad/causal mask) V) WO.

Sharding: 8 cores = 4 batches x 2 head-groups (Megatron-style). Each core computes,
for its batch b and its 8 heads:
    QT/KT = (X Wq_g)^T in [dh, seq] layout,  V = X Wv_g in [seq, dh] layout,
    S^T tiles [k, q] (so pad mask = per-partition ACT bias, no transposes anywhere),
    P = exp(S^T/8 + pad) (no max subtraction; scores are O(1)),
    causal handled by narrowing the q-range of diagonal-straddling tiles plus one
    128x128 triangular 0/1 multiply on the diagonal block,
    ctx^T (+rowsum via an all-ones 65th column of V) = Vaug^T @ P accumulated in PSUM,
    ctx^T normalized by 1/rowsum (reciprocal of the rowsum row, broadcast across
    partitions with a K=2 ones-matmul), then out_partial = ctx @ Wo_g.
Host sums the two head-group partials per batch. No collectives needed.

The emission is software-pipelined over q-stripes j: projection work for later
stripes and deferred output projections are interleaved between the attention
i-iterations, and scores for i+1 are emitted before the PV matmuls of i so the
TensorEngine never sits behind the exp latency.

Scheduling notes (from perfetto analysis):
  - the two per-head-pair scores matmuls (K=64) auto-row-tile via base
    partitions 0/64 and run CONCURRENTLY on the PE (pair ~= one N-cycle pass);
  - a short chain of throwaway warm-up matmuls runs while the first DMA
    chunks land, so the PE's DVFS ramp is already hot when real work arrives;
  - stripe 0 computes only the minimal prefix (Q slab 0, K slab 0, V) before
    attention starts; Q/K slabs 1-3 are force-popped at the hp block starts,
    so the first exp fires ~20us earlier than a full-projection prologue;
  - startup loads are chunked per-kt across THREE DMA trigger queues in strict
    consumption order (thin 128-col W slabs first); every steady-state load is
    also chunked so no 6us monolithic descriptor-gen ever blocks a queue;
  - interleave units are 2-matmul quarters paced by a fractional credit;
  - the norm's rowsum-gather/shift DMAs issue right at PSUM evacuation, the
    compute half (broadcast matmul, reciprocal, scale) is deferred, so the
    gathers' ~2us DMA latency is always covered by attention iterations;
  - the very last normalization replaces its DMAs with selector matmuls
    (rowsum broadcast + head-B shift on the PE) so the final WO block is not
    gated on SBUF-DMA latency at the tail;
  - WO fillers are deferred to the late, exp-throughput-bound stripes;
  - output staged bf16 (summed f32 host-side) to halve the final DMA drain.
"""

import numpy as np
import ml_dtypes

BF = ml_dtypes.bfloat16
S = 2048
D = 1024
HG = 512          # head-group width (8 heads x 64)
DH = 64
NKT = 16          # seq tiles of 128 (k side)
NQT = 4           # seq tiles of 512 (q side)
NEG = -30000.0

_CACHE = {}


def _build():
    import concourse.bass as bass  # noqa: F401
    import concourse.tile as tile
    from concourse import bacc, mybir

    f32 = mybir.dt.float32
    bf16 = mybir.dt.bfloat16
    Exp = mybir.ActivationFunctionType.Exp

    nc = bacc.Bacc("TRN2", target_bir_lowering=False, debug=False, num_devices=8)

    xq_d = nc.dram_tensor("xq", [D, S], bf16, kind="ExternalInput")
    xk_d = nc.dram_tensor("xk", [D, S], bf16, kind="ExternalInput")
    xv_d = nc.dram_tensor("xv", [D, S], bf16, kind="ExternalInput")
    wq_d = nc.dram_tensor("wq", [D, HG], bf16, kind="ExternalInput")
    wk_d = nc.dram_tensor("wk", [D, HG], bf16, kind="ExternalInput")
    wv_d = nc.dram_tensor("wv", [D, HG], bf16, kind="ExternalInput")
    wo_d = nc.dram_tensor("wo", [HG, D], bf16, kind="ExternalInput")
    padb_d = nc.dram_tensor("padb", [128, NKT], f32, kind="ExternalInput")
    cst_d = nc.dram_tensor("cst", [128, 704], bf16, kind="ExternalInput")
    out_d = nc.dram_tensor("out", [S, D], bf16, kind="ExternalOutput")

    with tile.TileContext(nc) as tc:
        with (
            tc.tile_pool(name="consts", bufs=1) as consts,
            tc.tile_pool(name="big", bufs=1) as big,
            tc.tile_pool(name="xpool", bufs=6) as xpool,
            tc.tile_pool(name="ppool", bufs=4) as ppool,
            tc.tile_pool(name="cspool", bufs=6) as cspool,
            tc.tile_pool(name="rspool", bufs=4) as rspool,
            tc.tile_pool(name="tmppool", bufs=3) as tmppool,
            tc.tile_pool(name="outpool", bufs=3) as outpool,
            tc.tile_pool(name="pspool", bufs=2, space="PSUM") as pspool,
            tc.tile_pool(name="psupool", bufs=2, space="PSUM") as psupool,
            tc.tile_pool(name="ctxpool", bufs=2, space="PSUM") as ctxpool,
        ):
            wq_sb = consts.tile([128, 8, HG], bf16, tag="wq")
            wk_sb = consts.tile([128, 8, HG], bf16, tag="wk")
            wv_sb = consts.tile([128, 8, HG], bf16, tag="wv")
            wo_sb = consts.tile([128, 4, D], bf16, tag="wo")
            padb_sb = consts.tile([128, NKT], f32, tag="padb")
            cst_sb = consts.tile([128, 704], bf16, tag="cst")
            trim_sb = cst_sb[:, 0:256].rearrange("p (d c) -> p d c", d=2)
            sel_sb = cst_sb[0:65, 256:576]
            ones2_sb = cst_sb[0:2, 576:704]
            scr_sb = consts.tile([128, 512], bf16, tag="scr")

            qt_sb = big.tile([128, 4, S], bf16, tag="qt")    # (X Wq)^T : rows = dh
            kt_sb = big.tile([128, 4, S], bf16, tag="kt")
            vaug_sb = big.tile([128, NKT, 8 * 65], bf16, tag="vaug")  # V + ones col
            ctxt_sb = big.tile([128, 4, S], bf16, tag="ctxt")
            vaug_h = vaug_sb.rearrange("p m (h e) -> p m h e", e=65)

            # ---- warm-up: ~10 throwaway matmuls keep the PE busy while the
            # first DMA chunks land, so the DVFS ramp is hot for real work
            nc.vector.memset(scr_sb, 0.25)
            nc.vector.memset(vaug_h[:, :, :, 64:65], 1.0)
            wup = psupool.tile([128, 512], f32, tag="psu")
            for _ in range(6):
                nc.tensor.matmul(wup, lhsT=scr_sb[:, 0:128], rhs=scr_sb,
                                 start=True, stop=True)

            def load_consts():
                nc.gpsimd.dma_start(out=padb_sb, in_=padb_d.ap())
                nc.gpsimd.dma_start(out=cst_sb, in_=cst_d.ap())

            def load_w_cols(dst, dram, kt, c0, c1, q):
                def emit():
                    q.dma_start(
                        out=dst[:, kt, c0:c1],
                        in_=dram.ap().rearrange(
                            "(kt p) n -> p kt n", p=128)[:, kt, c0:c1])
                return emit

            def load_wo_chunk(kt, q):
                def emit():
                    ktt, h = kt // 2, kt % 2
                    cols = slice(512 * h, 512 * (h + 1))
                    q.dma_start(
                        out=wo_sb[:, ktt, cols],
                        in_=wo_d.ap().rearrange(
                            "(kt p) n -> p kt n", p=128)[:, ktt, cols])
                return emit

            # ---------- x loads (chunked) + projections per q/seq stripe ----------
            x_tiles = {}

            def alloc_x(j, name):
                t = xpool.tile([128, 8, 512], bf16, tag="x")
                x_tiles[(j, name)] = t

            def load_x_2kt(j, name, dram, kt2, q):
                qs = slice(512 * j, 512 * (j + 1))

                def emit():
                    q.dma_start(
                        out=x_tiles[(j, name)][:, 2 * kt2:2 * kt2 + 2, :],
                        in_=dram.ap().rearrange(
                            "(kt p) s -> p kt s", p=128)[:, 2 * kt2:2 * kt2 + 2, qs])
                return emit

            def load_x_1kt(j, name, dram, kt, q):
                qs = slice(512 * j, 512 * (j + 1))

                def emit():
                    q.dma_start(
                        out=x_tiles[(j, name)][:, kt:kt + 1, :],
                        in_=dram.ap().rearrange(
                            "(kt p) s -> p kt s", p=128)[:, kt:kt + 1, qs])
                return emit

            def a_loads(j):
                # DMA issues only (no PE cost): emitted inline at the start of
                # the PREVIOUS stripe's attention so transfers overlap it
                alloc_x(j, "xq"); alloc_x(j, "xk"); alloc_x(j, "xv")
                units = []
                for kt2 in range(4):
                    units.append(load_x_2kt(j, "xq", xq_d, kt2,
                                            nc.sync if kt2 % 2 == 0 else nc.gpsimd))
                for kt2 in range(4):
                    units.append(load_x_2kt(j, "xk", xk_d, kt2,
                                            nc.gpsimd if kt2 % 2 == 0 else nc.sync))
                for kt2 in range(4):
                    units.append(load_x_2kt(j, "xv", xv_d, kt2,
                                            nc.sync if kt2 % 2 == 0 else nc.gpsimd))
                return units

            hold = {}

            def proj_t(w_sb, dst_sb, j, t, x_name, part):
                # 2-matmul quarters so the interleaver can fill the ~400ns
                # exp-latency bubbles; the PSUM tile carries across parts
                qs = slice(512 * j, 512 * (j + 1))

                def emit():
                    key = (j, x_name, t)
                    if part == 0:
                        ps_new = psupool.tile([128, 512], f32, tag="psu")
                        hold[key] = ps_new
                    ps = hold[key]
                    for kt in range(2 * part, 2 * part + 2):
                        nc.tensor.matmul(
                            ps,
                            lhsT=w_sb[:, kt, 128 * t:128 * (t + 1)],
                            rhs=x_tiles[(j, x_name)][:, kt, :],
                            start=(kt == 0), stop=(kt == 7),
                        )
                    if part == 3:
                        nc.vector.tensor_copy(out=dst_sb[:, t, qs], in_=ps)
                return emit

            def proj_v(j, m, part):
                def emit():
                    key = (j, "v", m)
                    if part == 0:
                        ps_new = psupool.tile([128, 512], f32, tag="psu")
                        hold[key] = ps_new
                    ps = hold[key]
                    for kt in range(2 * part, 2 * part + 2):
                        nc.tensor.matmul(
                            ps,
                            lhsT=x_tiles[(j, "xv")][:, kt, 128 * (m - 4 * j):128 * (m - 4 * j + 1)],
                            rhs=wv_sb[:, kt, :],
                            start=(kt == 0), stop=(kt == 7),
                        )
                    if part == 3:
                        nc.vector.tensor_copy(
                            out=vaug_h[:, m, :, 0:64],
                            in_=ps.rearrange("p (h e) -> p h e", e=64),
                        )
                return emit

            def a_compute(j):
                units = []
                for t in range(4):
                    for part in range(4):
                        units.append(proj_t(wq_sb, qt_sb, j, t, "xq", part))
                for t in range(4):
                    for part in range(4):
                        units.append(proj_t(wk_sb, kt_sb, j, t, "xk", part))
                for m in range(4 * j, 4 * j + 4):
                    for part in range(4):
                        units.append(proj_v(j, m, part))
                return units

            # ---------- stripe-C: output projection for q stripe j ----------
            def c_units(j):
                holder = {}

                def wo_mn(m, n, half):
                    def emit():
                        if half == 0:
                            ps_new = psupool.tile([128, 512], f32, tag="psu")
                            holder[(m, n)] = ps_new
                        ps = holder[(m, n)]
                        for kt in (0, 1) if half == 0 else (2, 3):
                            nc.tensor.matmul(
                                ps,
                                lhsT=ctxt_sb[:, kt, 128 * m:128 * (m + 1)],
                                rhs=wo_sb[:, kt, 512 * n:512 * (n + 1)],
                                start=(kt == 0), stop=(kt == 3),
                            )
                        if half == 1:
                            # bf16 staging halves the output bytes: the two
                            # per-core partials are summed in f32 on the host
                            o = outpool.tile([128, 512], bf16, tag="o")
                            nc.vector.tensor_copy(out=o, in_=ps)
                            nc.sync.dma_start(
                                out=out_d.ap()[128 * m:128 * (m + 1),
                                               512 * n:512 * (n + 1)], in_=o)
                    return emit
                return [wo_mn(m, n, half)
                        for m in range(4 * j, 4 * j + 4)
                        for n in range(2) for half in range(2)]

            # ---------- normalization ----------
            # phase 1 (at evacuation): rowsum-gather + head-B shift DMAs issue
            # immediately so their ~2us latency hides under later iterations.
            # phase 2 (deferred): broadcast matmul + reciprocal + scales.
            pending_norm = []

            def norm_phase1(cs_a, cs_b):
                rs = rspool.tile([2, 512], bf16, tag="rs")
                csb2 = tmppool.tile([128, 512], bf16, tag="tmp")
                nc.gpsimd.dma_start(out=rs[0:1, :], in_=cs_a[64:65, :])
                nc.gpsimd.dma_start(out=rs[1:2, :], in_=cs_b[64:65, :])
                nc.gpsimd.dma_start(out=csb2[64:128, :], in_=cs_b[0:64, :])
                return rs, csb2

            def make_norm2(cs_a, cs_b, rs, csb2, hp, j):
                qs = slice(512 * j, 512 * (j + 1))

                def emit():
                    bc = psupool.tile([128, 512], f32, tag="psu")
                    nc.tensor.matmul(bc, lhsT=ones2_sb, rhs=rs,
                                     start=True, stop=True)
                    rb = rspool.tile([128, 512], f32, tag="rb")
                    nc.vector.reciprocal_approx_fast(rb, bc)
                    nc.vector.tensor_mul(
                        ctxt_sb[0:64, hp, qs], cs_a[0:64, :], rb[0:64, :])
                    nc.vector.tensor_mul(
                        ctxt_sb[64:128, hp, qs], csb2[64:128, :], rb[64:128, :])
                return emit

            def norm2_tail(cs_a, cs_b, hp, j):
                # DMA-free variant for the very last normalization: rowsum
                # broadcast and head-B shift run as selector matmuls on the
                # PE so the final WO block is not gated on DMA latency
                qs = slice(512 * j, 512 * (j + 1))
                rbps = psupool.tile([128, 512], f32, tag="psu")
                nc.tensor.matmul(rbps, lhsT=sel_sb[:, 0:128], rhs=cs_a,
                                 start=True, stop=False)
                nc.tensor.matmul(rbps, lhsT=sel_sb[:, 128:256], rhs=cs_b,
                                 start=False, stop=True)
                csps = psupool.tile([128, 512], f32, tag="psu")
                nc.tensor.matmul(csps[64:128, :], lhsT=sel_sb[:, 256:320],
                                 rhs=cs_b, start=True, stop=True)
                rb = rspool.tile([128, 512], f32, tag="rb")
                nc.vector.reciprocal_approx_fast(rb, rbps)
                nc.vector.tensor_mul(
                    ctxt_sb[0:64, hp, qs], cs_a[0:64, :], rb[0:64, :])
                nc.vector.tensor_mul(
                    ctxt_sb[64:128, hp, qs], csps[64:128, :], rb[64:128, :])

            # ---------- stripe-B: attention for q stripe j, with interleave ----------
            def b_stripe(j, queue, hp_force=0, i_force=0):
                ni = 4 * j + 4
                tot_slots = 2 * 4 * ni
                pace = len(queue) / tot_slots if queue else 0.0
                credit = 0.0

                def lo(i):
                    return max(0, (i - 4 * j) * 128)

                def pop_filler(n):
                    for _ in range(n):
                        if pending_norm and queue and queue[0][0] == "c":
                            pending_norm.pop(0)()
                        elif queue:
                            queue.pop(0)[1]()
                        else:
                            break

                for hp in range(4):
                    if hp > 0 and hp_force:
                        # stripe 0: Q/K slab hp must be projected before this
                        # hp block's scores read it (in-order PE queue)
                        pop_filler(hp_force)
                    h0, h1 = 2 * hp, 2 * hp + 1
                    ctx_a = ctxpool.tile([65, 512], f32, tag="ctx")
                    ctx_b = ctxpool.tile([65, 512], f32, tag="ctx")
                    sps = {}

                    def scores(i):
                        c = lo(i)
                        ks = slice(128 * i, 128 * (i + 1))
                        sp = pspool.tile([128, 2, 512], f32, tag="ps")
                        nc.tensor.matmul(
                            sp[:, 0, c:], lhsT=kt_sb[0:64, hp, ks],
                            rhs=qt_sb[0:64, hp, 512 * j + c:512 * (j + 1)],
                            start=True, stop=True)
                        nc.tensor.matmul(
                            sp[:, 1, c:], lhsT=kt_sb[64:128, hp, ks],
                            rhs=qt_sb[64:128, hp, 512 * j + c:512 * (j + 1)],
                            start=True, stop=True)
                        sps[i] = sp

                    scores(0)
                    for i in range(ni):
                        if i + 1 < ni:
                            scores(i + 1)
                        c = lo(i)
                        sp = sps.pop(i)
                        p = ppool.tile([128, 2, 512], bf16, tag="p")
                        nc.scalar.activation(
                            out=p[:, :, c:], in_=sp[:, :, c:], func=Exp,
                            bias=padb_sb[:, i:i + 1], scale=0.125)
                        if i >= 4 * j:
                            nc.vector.tensor_mul(
                                p[:, :, c:c + 128], p[:, :, c:c + 128], trim_sb)

                        def insert_filler(force=False):
                            nonlocal credit
                            credit += pace
                            if force and credit < 1.0 and queue:
                                credit = 1.0
                            while credit >= 1.0 and queue:
                                credit -= 1.0
                                pop_filler(1)

                        if i == 3 and pending_norm:
                            pending_norm.pop(0)()
                        if i_force and hp == 0:
                            # stripe 0: V slab i must be projected before this
                            # iteration's PV matmuls read it
                            pop_filler(i_force)
                        insert_filler(force=(i == 0))
                        nc.tensor.matmul(
                            ctx_a[:, c:], lhsT=vaug_sb[:, i, 65 * h0:65 * h0 + 65],
                            rhs=p[:, 0, c:],
                            start=(i == 0), stop=(i == ni - 1))
                        nc.tensor.matmul(
                            ctx_b[:, c:], lhsT=vaug_sb[:, i, 65 * h1:65 * h1 + 65],
                            rhs=p[:, 1, c:],
                            start=(i == 0), stop=(i == ni - 1))
                        insert_filler()
                    # evacuate PSUM immediately (frees the bank for hp+1)
                    cs_a = cspool.tile([65, 512], bf16, tag="cs")
                    nc.vector.tensor_copy(out=cs_a, in_=ctx_a)
                    cs_b = cspool.tile([65, 512], bf16, tag="cs")
                    nc.vector.tensor_copy(out=cs_b, in_=ctx_b)
                    if j == 3 and hp == 3:
                        norm2_tail(cs_a, cs_b, hp, j)
                    else:
                        rs, csb2 = norm_phase1(cs_a, cs_b)
                        pending_norm.append(make_norm2(cs_a, cs_b, rs, csb2, hp, j))

            # ---------- emit the pipeline ----------
            # DMA trigger queues: sync + gpsimd ONLY. The scalar queue must
            # carry nothing but exp: its in-order issue would otherwise gate
            # the first ACTIVATE behind ring-credit waits for ~10us.
            qs2 = [nc.sync, nc.gpsimd]
            qi = 0

            def q3():
                nonlocal qi
                q = qs2[qi % 2]
                qi += 1
                return q

            # stripe-0 loads, strict consumption order: thin W col-slabs for
            # the t0 projections first, then the full V/W tails
            alloc_x(0, "xq"); alloc_x(0, "xk"); alloc_x(0, "xv")
            for kt in range(8):
                load_w_cols(wq_sb, wq_d, kt, 0, 128, q3())()
            for kt2 in range(4):
                load_x_2kt(0, "xq", xq_d, kt2, q3())()
            for kt in range(8):
                load_w_cols(wk_sb, wk_d, kt, 0, 128, q3())()
            for kt2 in range(4):
                load_x_2kt(0, "xk", xk_d, kt2, q3())()
            load_consts()
            for kt2 in range(4):
                load_x_2kt(0, "xv", xv_d, kt2, q3())()
                load_w_cols(wv_sb, wv_d, 2 * kt2, 0, HG, q3())()
                load_w_cols(wv_sb, wv_d, 2 * kt2 + 1, 0, HG, q3())()
            for kt in range(8):
                load_w_cols(wq_sb, wq_d, kt, 128, HG, q3())()
                load_w_cols(wk_sb, wk_d, kt, 128, HG, q3())()
            for kt in range(8):
                load_wo_chunk(kt, q3())()

            # stripe-0 minimal compute prefix: Q slab 0 and K slab 0 only --
            # everything else (V for PV, Q/K slabs 1-3 for hp1-3) is deferred
            # into the stripe-0 queue and force-popped just in time, so hp0's
            # scores/exp run while the V data is still in flight
            for part in range(4):
                proj_t(wq_sb, qt_sb, 0, 0, "xq", part)()
            for part in range(4):
                proj_t(wk_sb, kt_sb, 0, 0, "xk", part)()

            s0_rest = []
            for m in range(4):
                for part in range(4):
                    s0_rest.append(("a", proj_v(0, m, part)))
            for t in range(1, 4):
                for part in range(4):
                    s0_rest.append(("a", proj_t(wq_sb, qt_sb, 0, t, "xq", part)))
                for part in range(4):
                    s0_rest.append(("a", proj_t(wk_sb, kt_sb, 0, t, "xk", part)))

            filler = {}
            for jn in (1, 2, 3):
                filler[jn - 1] = [("a", u) for u in a_compute(jn)]
            filler[0] = s0_rest + filler[0]
            filler[3] = [("c", u) for u in c_units(0)] + \
                        [("c", u) for u in c_units(1)] + \
                        [("c", u) for u in c_units(2)]

            for j in range(NQT):
                if j < NQT - 1:
                    for u in a_loads(j + 1):
                        u()
                queue = filler[j]
                b_stripe(j, queue, hp_force=(8 if j == 0 else 0),
                         i_force=(4 if j == 0 else 0))
                if j < NQT - 1:
                    while pending_norm and len(pending_norm) > 1:
                        pending_norm.pop(0)()
                    for _, u in queue:
                        u()
                else:
                    # tail: a few leftover fillers keep the PE hot while the
                    # inline hp3 norm completes, then the last WO drains
                    leftover = list(queue)
                    while pending_norm:
                        pending_norm.pop(0)()
                    for _ in range(4):
                        if leftover:
                            leftover.pop(0)[1]()
                    c3 = c_units(3)
                    for idx, u in enumerate(c3):
                        u()
                        if idx % 2 == 1 and leftover:
                            leftover.pop(0)[1]()
                    for _, u in leftover:
                        u()

    nc.compile()
    return nc


def _make_trim():
    p = np.arange(128)[:, None]
    f = np.arange(128)[None, :]
    return (f >= p).astype(np.float32).astype(BF)


def kernel(Q_emb, K_emb, V_emb, Q_ini, K_ini, WQ, WK, WV, WO):
    from concourse.bass_utils import run_bass_kernel_spmd

    if "nc" not in _CACHE:
        _CACHE["nc"] = _build()
    nc = _CACHE["nc"]

    Q_emb = np.asarray(Q_emb, np.float32)
    K_emb = np.asarray(K_emb, np.float32)
    V_emb = np.asarray(V_emb, np.float32)
    K_ini = np.asarray(K_ini)
    WQ = np.asarray(WQ, np.float32)
    WK = np.asarray(WK, np.float32)
    WV = np.asarray(WV, np.float32)
    WO = np.asarray(WO, np.float32)

    # packed bf16 consts: [trim x2 | sel | ones2].
    # sel = [eA | eB | shift] for the DMA-free tail normalization:
    # eA broadcasts cs_a row 64 to out rows 0:64, eB broadcasts cs_b row 64
    # to out rows 64:128, shift moves cs_b rows 0:64 to out rows 64:128.
    trim = _make_trim().astype(np.float32)
    cst = np.zeros((128, 704), np.float32)
    cst[:, 0:128] = trim
    cst[:, 128:256] = trim
    cst[64, 256:320] = 1.0
    cst[64, 448:512] = 1.0
    cst[np.arange(64), 512 + np.arange(64)] = 1.0
    cst[0, 576:640] = 1.0
    cst[1, 640:704] = 1.0
    cst = cst.astype(BF)
    in_maps = []
    for c in range(8):
        b, g = c // 2, c % 2
        gs = slice(HG * g, HG * (g + 1))
        padb = np.where(K_ini[b] != 0, 0.0, NEG).astype(np.float32)
        if padb[0] != 0.0:
            # key 0 masked would make causal row 0 fully masked -> rowsum 0 ->
            # NaN. The reference emits an (arbitrary) softmax over masked
            # scores there; keep key 0 live so output stays finite.
            padb[0] = 0.0
        in_maps.append({
            "xq": Q_emb[b].T.astype(BF),
            "xk": K_emb[b].T.astype(BF),
            "xv": V_emb[b].T.astype(BF),
            "wq": WQ[:, gs].astype(BF),
            "wk": WK[:, gs].astype(BF),
            "wv": WV[:, gs].astype(BF),
            "wo": WO[gs, :].astype(BF),
            "padb": padb.reshape(NKT, 128).T.copy(),
            "cst": cst,
        })

    _CACHE["in_maps"] = in_maps
    res = run_bass_kernel_spmd(nc, in_maps, list(range(8)))
    parts = [res.results[c]["out"].astype(np.float32) for c in range(8)]
    out = np.stack([parts[2 * b] + parts[2 * b + 1] for b in range(4)])
    return out.astype(np.float32)
